# revision 3
# baseline (speedup 1.0000x reference)
"""Multi-head causal attention (B=2, N=2048, D=1024, H=16) on 8 Trainium2 cores.

v2 rewrite of the tensor-parallel-over-heads kernel, aimed at keeping the PE
continuously busy (p-state ramp) and spreading non-matmul work across the
Activation/DVE/Pool engines:

  - work unit = 512 q tokens (8 units); per unit: scores -> exp -> AV per
    128-wide key block, accumulated into per-head [65, 512] PSUM (ones column
    of V carries the softmax denominator).
  - scores: both heads' 64-contraction matmuls issued back-to-back at PE row
    halves (tile_position) into one [128, 1024] PSUM tile (head halves);
    ONE exp ACTIVATE covers both heads via a 3D AP (suffix width only).
  - V is projected directly token-major (stationary x-tile, streamed Wv):
    no PE transposes, single psum->SBUF copy per token block.
  - normalize: DVE reciprocal of the denominator row (read straight from
    PSUM), Pool partition_broadcast, DVE multiply -> bf16 staging tile.
  - exchange: one staging DMA per unit + per-unit AllToAll; output projection
    runs per section (2 units), one unit late, so collectives hide under
    compute; the final section's projection is split per 64-token half.
"""

import os

import numpy as np
import ml_dtypes

DBG = os.environ.get("K_DBG", "0") == "1"

from concourse import bacc, tile, mybir
import concourse.bass as bass
from concourse.bass_utils import run_bass_kernel_spmd

NCORES = 8
B, N, D, H, HD = 2, 2048, 1024, 16, 64
TOK = B * N              # 4096
HPC = H // NCORES        # 2 heads per core
TPC = TOK // NCORES      # 512 output tokens per core
BT = 128                 # key block size
UW = 512                 # unit q width
NU = TOK // UW           # 8 units
CH = 512                 # token chunk for QKV projection matmuls
NBU = N // BT            # 16 key blocks per batch
F32 = mybir.dt.float32
BF16 = mybir.dt.bfloat16
EXP = mybir.ActivationFunctionType.Exp
BF = ml_dtypes.bfloat16


def make_plan(mask):
    """Per-unit key-block schedule derived from the [1,1,N,N] mask.

    units[u] = {kb: {"c0": first valid local q col (128-aligned),
                     "pats": [(qb_local, pattern_idx), ...]}}
    """
    m = np.asarray(mask).reshape(N, N)
    patterns = []
    pat_keys = {}
    units = []
    for u in range(NU):
        q0 = (u % 4) * UW
        sched = {}
        for kb in range(NBU):
            blk = m[q0:q0 + UW, kb * BT:(kb + 1) * BT]  # [512 q, 128 k]
            if not blk.any():
                continue
            rows = blk.any(axis=1)
            c0 = (int(np.argmax(rows)) // BT) * BT
            pats = []
            for qb in range(c0 // BT, UW // BT):
                b2 = blk[qb * BT:(qb + 1) * BT, :]
                if b2.all() or not b2.any():
                    continue
                pat = np.ascontiguousarray(b2.T.astype(np.float32))
                key = pat.tobytes()
                if key not in pat_keys:
                    pat_keys[key] = len(patterns)
                    patterns.append(pat)
                pats.append((qb, pat_keys[key]))
            sched[kb] = {"c0": c0, "pats": pats}
        units.append(sched)
    if not patterns:
        patterns.append(np.ones((BT, BT), np.float32))
    return {"patterns": np.stack(patterns).astype(BF), "units": units}


def build_nc(plan):
    nc = bacc.Bacc("TRN2", target_bir_lowering=False, debug=False,
                   num_devices=NCORES)
    n_pat = plan["patterns"].shape[0]
    units = plan["units"]

    xP = nc.dram_tensor("xP", [8 * BT, 8 * CH], BF16, kind="ExternalInput")
    wqP = nc.dram_tensor("wqP", [BT, 8 * BT], BF16, kind="ExternalInput")
    wkP = nc.dram_tensor("wkP", [BT, 8 * BT], BF16, kind="ExternalInput")
    wvP = nc.dram_tensor("wvP", [BT, 8 * BT], BF16, kind="ExternalInput")
    woP = nc.dram_tensor("woP", [BT, 8 * D], BF16, kind="ExternalInput")
    bo1 = nc.dram_tensor("bo1", [1, D], F32, kind="ExternalInput")
    pm = nc.dram_tensor("pm", [n_pat, BT, BT], BF16, kind="ExternalInput")
    ident = nc.dram_tensor("ident", [BT, BT], F32, kind="ExternalInput")
    y = nc.dram_tensor("y", [TPC, D], F32, kind="ExternalOutput")
    if DBG:
        d_qT = nc.dram_tensor("d_qT", [BT, TOK], BF16, kind="ExternalOutput")
        d_kT = nc.dram_tensor("d_kT", [BT, TOK], BF16, kind="ExternalOutput")
        d_v2 = nc.dram_tensor("d_v2", [BT, 32 * 130], BF16,
                              kind="ExternalOutput")
        d_pT = nc.dram_tensor("d_pT", [BT, 1024], BF16, kind="ExternalOutput")
        d_pO = nc.dram_tensor("d_pO", [HD + 1, UW], F32,
                              kind="ExternalOutput")
        d_aS = nc.dram_tensor("d_aS", [BT, UW], BF16, kind="ExternalOutput")
        d_aT = nc.dram_tensor("d_aT", [BT, 8 * BT], BF16,
                              kind="ExternalOutput")

    with tile.TileContext(nc) as tc:
        with (
            tc.tile_pool(name="const", bufs=1) as cp,
            tc.tile_pool(name="big", bufs=1) as bigp,
            tc.tile_pool(name="psum", bufs=1, space="PSUM") as psum,
            tc.tile_pool(name="dram", bufs=1, space="DRAM") as dram,
        ):
            # weight/const tiles (DMA issue order is tuned below: wq first,
            # then the first x chunk, then the rest)
            wq_all = cp.tile([BT, 8 * BT], BF16, name="wq_all")
            wk_all = cp.tile([BT, 8 * BT], BF16, name="wk_all")
            wv_all = cp.tile([BT, 8 * BT], BF16, name="wv_all")
            wq = [wq_all[:, e * BT:(e + 1) * BT] for e in range(8)]
            wk = [wk_all[:, e * BT:(e + 1) * BT] for e in range(8)]
            wv = [wv_all[:, e * BT:(e + 1) * BT] for e in range(8)]
            wo_all = cp.tile([BT, 8 * D], BF16, name="wo_all")
            pmt = [cp.tile([BT, BT], BF16, name=f"pmt{i}") for i in range(n_pat)]
            identt = cp.tile([BT, BT], F32, name="identt")
            bot = cp.tile([1, D], F32, name="bot")
            bobc = cp.tile([BT, D], F32, name="bobc")

            # ---- warm-up collective, fired immediately: the CC stream's
            # init barrier completes only once ALL cores reach their first
            # trigger, so firing at t~2us absorbs the core-launch skew
            # before the real exchanges need the stream ----
            wa_sb = cp.tile([BT, 4], F32, name="wa_sb")
            nc.vector.memset(wa_sb[:], 1.0)
            wa_in = dram.tile([BT, 4], F32, name="wa_in")
            wa_out = dram.tile([BT * NCORES, 4], F32, name="wa_out",
                               addr_space="Shared")
            nc.gpsimd.dma_start(wa_in[:], wa_sb[:])
            nc.gpsimd.collective_compute(
                "AllGather", mybir.AluOpType.bypass,
                ins=[wa_in.opt()], outs=[wa_out.opt()],
                replica_groups=[list(range(NCORES))])

            # A collective trigger freezes the Pool queue until the CC
            # stream accepts it. Early triggers are deferred one unit so
            # the Pool-hosted reciprocal broadcasts of units 1-3 are not
            # frozen behind them while the stream warms up.
            def trigger_a2a(u):
                nc.gpsimd.collective_compute(
                    "AllToAll", mybir.AluOpType.bypass,
                    ins=[a2a_in[u].opt()], outs=[a2a_out[u].opt()],
                    replica_groups=[list(range(NCORES))])

            # per-unit exchange buffers
            a2a_in = [dram.tile([NCORES * BT, HD], BF16, name=f"a2a_in{u}")
                      for u in range(NU)]
            a2a_out = [dram.tile([NCORES * BT, HD], BF16, name=f"a2a_out{u}")
                       for u in range(NU)]

            qTt = bigp.tile([BT, TOK], BF16, name="qTt")
            kTt = bigp.tile([BT, TOK], BF16, name="kTt")
            # V token-major: [128 tok, 130] per block: [64 h0 | 1 | 64 h1 | 1]
            v2 = bigp.tile([BT, 32 * 130], BF16, name="v2")
            nc.vector.memset(
                v2[:].rearrange("p (t j c) -> p t j c", t=32, j=2)
                [:, :, :, HD:HD + 1], 1.0)

            with (
                tc.tile_pool(name="xp", bufs=2) as xp,
                tc.tile_pool(name="ptp", bufs=3) as ptp,
                tc.tile_pool(name="workp", bufs=2) as workp,
            ):
                xtiles = {}

                def load_x(ch):
                    xt_all = xp.tile([BT, 8 * CH], BF16, name="xt", tag="xt")
                    for q in range(4):
                        cs = slice(1024 * q, 1024 * (q + 1))
                        nc.sync.dma_start(
                            xt_all[:, cs],
                            xP.ap()[ch * BT:(ch + 1) * BT, cs])
                    xtiles[ch] = xt_all

                def emit_qkv_chunk(ch):
                    xt_all = xtiles.pop(ch)
                    xt = [xt_all[:, e * CH:(e + 1) * CH] for e in range(8)]
                    # Q and K into one [128, 1024] PSUM tile (psS tag)
                    psqk = psum.tile([BT, 1024], F32, name="psS", tag="psS",
                                     bufs=2)
                    for (qk, wt) in ((0, wq), (1, wk)):
                        for e in range(8):
                            nc.tensor.matmul(psqk[:, 512 * qk:512 * (qk + 1)],
                                             wt[e], xt[e],
                                             start=(e == 0), stop=(e == 7))
                    for (qk, dst) in ((0, qTt), (1, kTt)):
                        nc.vector.tensor_copy(dst[:, ch * CH:(ch + 1) * CH],
                                              psqk[:, 512 * qk:512 * (qk + 1)])
                    # V channel-major (512-wide streams), then token-major
                    # via PE transposes (bf16, 1 cycle/row)
                    psv = psum.tile([BT, 1024], F32, name="psS", tag="psS",
                                    bufs=2)
                    for e in range(8):
                        nc.tensor.matmul(psv[:, 0:CH], wv[e], xt[e],
                                         start=(e == 0), stop=(e == 7))
                    vTs = workp.tile([BT, CH], F32, name="vTs", tag="vTs")
                    nc.vector.tensor_copy(vTs[:], psv[:, 0:CH])
                    pst = psum.tile([BT, 1024], F32, name="psS", tag="psS",
                                    bufs=2)
                    for t in range(4):
                        nc.tensor.transpose(pst[:, t * BT:(t + 1) * BT],
                                            vTs[:, t * BT:(t + 1) * BT],
                                            identt[:])
                    for t in range(4):
                        tb = ch * 4 + t
                        nc.vector.tensor_copy(
                            v2[:, 130 * tb:130 * tb + 130]
                            .rearrange("p (j c) -> p j c", j=2)[:, :, 0:HD],
                            pst[:, t * BT:(t + 1) * BT]
                            .rearrange("p (j c) -> p j c", j=2))

                def emit_exchange(u, attnS):
                    # a2a_in[(r, jp), c] = attnS[jp, 64 r + c]
                    nc.sync.dma_start(
                        a2a_in[u][:].rearrange("(r p) c -> p r c", p=BT),
                        attnS[:].rearrange("p (r c) -> p r c", r=NCORES))
                    if u == 1:
                        trigger_a2a(0)
                    elif u == 2:
                        trigger_a2a(1)
                    elif u == 3:
                        trigger_a2a(2)
                        trigger_a2a(3)
                    elif u >= 4:
                        trigger_a2a(u)

                def emit_outproj(sec, halves=(0, 1), rows=None):
                    # aT[p, 128 i + 64 h + c] = a2a_out[2 sec + h][(i, p), c]
                    aT = workp.tile([BT, 8 * BT], BF16, name=f"aT{halves[0]}",
                                    tag=f"aT{halves[0]}")
                    for h in halves:
                        nc.sync.dma_start(
                            aT[:].rearrange("p (i g c) -> p i g c", i=8, g=2,
                                            c=HD)[:, :, h, :],
                            a2a_out[2 * sec + h][:]
                            .rearrange("(i p) c -> p i c", p=BT))
                    if DBG and sec == 0 and halves == (0, 1):
                        nc.sync.dma_start(d_aT.ap(), aT[:])
                    rs = rows if rows is not None else slice(0, BT)
                    aTs = [aT[:, i * BT:(i + 1) * BT][:, rs] for i in range(8)]
                    nr = rs.stop - rs.start
                    psY = psum.tile([BT, 1024], F32, name="psS", tag="psS",
                                    bufs=2)
                    for oc in range(2):
                        for i in range(8):
                            nc.tensor.matmul(
                                psY[0:nr, oc * 512:(oc + 1) * 512], aTs[i],
                                wo_all[:, i * D + oc * 512:
                                       i * D + (oc + 1) * 512],
                                start=(i == 0), stop=(i == 7))
                    for oc in range(2):
                        ysb = workp.tile([BT, 512], F32, name="ysb", tag="ysb")
                        nc.vector.tensor_add(
                            ysb[0:nr, :], psY[0:nr, oc * 512:(oc + 1) * 512],
                            bobc[rs, oc * 512:(oc + 1) * 512])
                        nc.sync.dma_start(
                            y.ap()[BT * sec + rs.start:BT * sec + rs.stop,
                                   oc * 512:(oc + 1) * 512],
                            ysb[0:nr, :])

                nc.scalar.dma_start(wq_all[:], wqP.ap())
                load_x(0)
                nc.scalar.dma_start(wk_all[:], wkP.ap())
                nc.scalar.dma_start(wv_all[:], wvP.ap())
                for i in range(n_pat):
                    nc.scalar.dma_start(pmt[i][:], pm.ap()[i])
                nc.scalar.dma_start(identt[:], ident.ap())
                nc.scalar.dma_start(bot[:], bo1.ap())
                nc.gpsimd.partition_broadcast(bobc[:], bot[:])
                for q in range(4):
                    cs = slice(2048 * q, 2048 * (q + 1))
                    nc.scalar.dma_start(wo_all[:, cs], woP.ap()[:, cs])

                for u in range(NU):
                    b, qh = u // 4, u % 4
                    if u + 1 < NU:
                        load_x(u + 1)
                    emit_qkv_chunk(u)

                    sched = units[u]
                    kbs = sorted(sched.keys())
                    psO = [psum.tile([HD + 1, UW], F32, name=f"psO{j}",
                                     tag=f"psO{j}", bufs=2)
                           for j in range(HPC)]
                    qcols = slice(b * N + qh * UW, b * N + (qh + 1) * UW)
                    diag = [kb for kb in kbs if sched[kb]["pats"]
                            or sched[kb]["c0"] > 0]
                    full = [kb for kb in kbs if kb not in diag]

                    def emit_scores(kb, tag):
                        kcols = slice(b * N + kb * BT, b * N + (kb + 1) * BT)
                        c0 = sched[kb]["c0"]
                        psS = psum.tile([BT, 1024], F32, name="psS", tag="psS",
                                        bufs=2)
                        pT = ptp.tile([BT, 1024], BF16, name="pT", tag=tag,
                                      bufs=4 if tag == "pTd" else 3)
                        for j in range(HPC):
                            ks = slice(HD * j, HD * (j + 1))
                            nc.tensor.matmul(
                                psS[:, 512 * j + c0:512 * (j + 1)],
                                kTt[ks, kcols],
                                qTt[ks, qcols][:, c0:UW],
                                start=True, stop=True,
                                tile_position=(HD * j, 0))
                        # one exp for both heads (3D AP, suffix width)
                        nc.scalar.activation(
                            pT[:].rearrange("p (j c) -> p j c",
                                            j=2)[:, :, c0:UW],
                            psS[:].rearrange("p (j c) -> p j c",
                                             j=2)[:, :, c0:UW],
                            EXP, scale=0.125)
                        # partial-block masking (per head, on DVE)
                        for (qb, pidx) in sched[kb]["pats"]:
                            for j in range(HPC):
                                slc = pT[:, 512 * j + qb * BT:
                                         512 * j + (qb + 1) * BT]
                                nc.vector.tensor_mul(slc, slc, pmt[pidx][:])
                        if DBG and u == 1 and kb == 0:
                            nc.sync.dma_start(d_pT.ap(), pT[:])
                        return pT

                    def emit_av(kb, pT, start, stop):
                        c0 = sched[kb]["c0"]
                        for j in range(HPC):
                            vt = v2[:, 130 * (b * NBU + kb) + 65 * j:
                                    130 * (b * NBU + kb) + 65 * (j + 1)]
                            nc.tensor.matmul(
                                psO[j][:, c0:UW], vt,
                                pT[:, 512 * j + c0:512 * (j + 1)],
                                start=start, stop=stop,
                                skip_group_check=True)

                    if full:
                        # diag scores first (their exp+mask pipeline runs
                        # while the PE chews the full blocks), diag AV last
                        dpts = [emit_scores(kb, "pTd") for kb in diag]
                        for (n, kb) in enumerate(full):
                            pT = emit_scores(kb, "pT")
                            emit_av(kb, pT, start=(n == 0), stop=False)
                        for (n, kb) in enumerate(diag):
                            emit_av(kb, dpts[n], start=False,
                                    stop=(n == len(diag) - 1))
                    else:
                        # first unit of a batch: all blocks diagonal
                        for (n, kb) in enumerate(diag):
                            pT = emit_scores(kb, "pT")
                            emit_av(kb, pT, start=(n == 0),
                                    stop=(n == len(diag) - 1))

                    if u == NU - 1:
                        # final section's first-half projection: emitted
                        # before the last staging so its unstage DMA is not
                        # queued behind it on the sync queue, and the PE
                        # fills the last collective's window
                        emit_outproj(3, halves=(0,), rows=slice(0, HD))

                    # normalize + stage + exchange
                    if DBG and u == 1:
                        pOsb = workp.tile([HD + 1, UW], F32, name="pOsb",
                                          tag="pOsb")
                        nc.vector.tensor_copy(pOsb[:], psO[0][:])
                        nc.sync.dma_start(d_pO.ap(), pOsb[:])
                    # copy unnormalized attention out first (releases psO
                    # without waiting on the Pool-hosted broadcast), then
                    # scale in place
                    attnS = workp.tile([BT, UW], BF16, name="attnS",
                                       tag="attnS")
                    for j in range(HPC):
                        dn = workp.tile([1, UW], F32, name=f"dn{j}",
                                        tag=f"dn{j}")
                        nc.vector.tensor_copy(dn[:], psO[j][HD:HD + 1, :])
                        rcp = workp.tile([1, UW], F32, name=f"rcp{j}",
                                         tag=f"rcp{j}")
                        nc.vector.reciprocal_approx_fast(rcp[:], dn[:])
                        rbc = workp.tile([HD, UW], F32, name=f"rbc{j}",
                                         tag=f"rbc{j}")
                        nc.gpsimd.partition_broadcast(rbc[:], rcp[:])
                        nc.vector.tensor_mul(attnS[HD * j:HD * (j + 1), :],
                                             psO[j][0:HD, :], rbc[:])
                    if DBG and u == 1:
                        nc.sync.dma_start(d_aS.ap(), attnS[:])
                    emit_exchange(u, attnS)
                    if u == 4:
                        emit_outproj(0)
                    elif u == 5:
                        emit_outproj(1)
                    elif u == 6:
                        emit_outproj(2)

                emit_outproj(3, halves=(1,), rows=slice(HD, BT))
                if DBG:
                    nc.sync.dma_start(d_qT.ap(), qTt[:])
                    nc.sync.dma_start(d_kT.ap(), kTt[:])
                    nc.sync.dma_start(d_v2.ap(), v2[:])
    nc.compile()
    return nc


_CACHE = {}


def _get_nc(plan_key, mask):
    if plan_key not in _CACHE:
        _CACHE[plan_key] = build_nc(make_plan(mask))
    return _CACHE[plan_key]


def _sb_layout(wT):
    """[8*128, C] row-block-major -> [128, 8*C] SBUF layout (contiguous)."""
    g, c = wT.shape[0] // BT, wT.shape[1]
    return np.ascontiguousarray(
        wT.reshape(g, BT, c).transpose(1, 0, 2).reshape(BT, g * c))


def _prep_inputs(x, mask, Wq, Wk, Wv, Wo, bo, plan):
    xT = x.reshape(TOK, D).T.astype(BF)          # [1024, 4096]
    xP = np.ascontiguousarray(
        xT.reshape(8, BT, 8, CH).transpose(2, 1, 0, 3).reshape(8 * BT, 8 * CH))
    woT = np.ascontiguousarray(Wo.T).astype(BF)
    bo1 = np.ascontiguousarray(bo[None, :]).astype(np.float32)
    woP = _sb_layout(woT)
    in_maps = []
    for c in range(NCORES):
        rows = slice(BT * c, BT * (c + 1))
        in_maps.append({
            "xP": xP,
            "wqP": _sb_layout(Wq[rows].T.astype(BF)),
            "wkP": _sb_layout(Wk[rows].T.astype(BF)),
            "wvP": _sb_layout(Wv[rows].T.astype(BF)),
            "woP": woP,
            "bo1": bo1,
            "pm": plan["patterns"],
            "ident": np.eye(BT, dtype=np.float32),
        })
    return in_maps


def run(inputs, trace=False, **kw):
    x = np.asarray(inputs["x"], np.float32)
    mask = np.asarray(inputs["mask"])
    plan_key = mask.tobytes()
    nc = _get_nc(plan_key, mask)
    plan = make_plan(mask)
    in_maps = _prep_inputs(x, mask, np.asarray(inputs["Wq"], np.float32),
                           np.asarray(inputs["Wk"], np.float32),
                           np.asarray(inputs["Wv"], np.float32),
                           np.asarray(inputs["Wo"], np.float32),
                           np.asarray(inputs["bo"], np.float32), plan)
    res = run_bass_kernel_spmd(nc, in_maps, core_ids=list(range(NCORES)),
                               trace=trace, **kw)
    # unshard: core c, section sec, half h, row i ->
    #   global token sec*1024 + 512 h + 64 c + i
    out = np.empty((TOK, D), np.float32)
    for c in range(NCORES):
        yc = res.results[c]["y"]
        for sec in range(4):
            for h in range(2):
                out[sec * 1024 + 512 * h + HD * c:
                    sec * 1024 + 512 * h + HD * (c + 1)] = \
                    yc[BT * sec + HD * h: BT * sec + HD * (h + 1)]
    return out.reshape(B, N, D), res


def kernel(**inputs):
    out, _ = run(inputs, trace=False)
    return out


# revision 4
# speedup vs baseline: 1.0038x; 1.0038x over previous
"""Multi-head causal attention (B=2, N=2048, D=1024, H=16) on 8 Trainium2 cores.

v2 rewrite of the tensor-parallel-over-heads kernel, aimed at keeping the PE
continuously busy (p-state ramp) and spreading non-matmul work across the
Activation/DVE/Pool engines:

  - work unit = 512 q tokens (8 units); per unit: scores -> exp -> AV per
    128-wide key block, accumulated into per-head [65, 512] PSUM (ones column
    of V carries the softmax denominator).
  - scores: both heads' 64-contraction matmuls issued back-to-back at PE row
    halves (tile_position) into one [128, 1024] PSUM tile (head halves);
    ONE exp ACTIVATE covers both heads via a 3D AP (suffix width only).
  - V is projected directly token-major (stationary x-tile, streamed Wv):
    no PE transposes, single psum->SBUF copy per token block.
  - normalize: DVE reciprocal of the denominator row (read straight from
    PSUM), Pool partition_broadcast, DVE multiply -> bf16 staging tile.
  - exchange: one staging DMA per unit + per-unit AllToAll; output projection
    runs per section (2 units), one unit late, so collectives hide under
    compute; the final section's projection is split per 64-token half.
"""

import os

import numpy as np
import ml_dtypes

DBG = os.environ.get("K_DBG", "0") == "1"

from concourse import bacc, tile, mybir
import concourse.bass as bass
from concourse.bass_utils import run_bass_kernel_spmd

NCORES = 8
B, N, D, H, HD = 2, 2048, 1024, 16, 64
TOK = B * N              # 4096
HPC = H // NCORES        # 2 heads per core
TPC = TOK // NCORES      # 512 output tokens per core
BT = 128                 # key block size
UW = 512                 # unit q width
NU = TOK // UW           # 8 units
CH = 512                 # token chunk for QKV projection matmuls
NBU = N // BT            # 16 key blocks per batch
F32 = mybir.dt.float32
BF16 = mybir.dt.bfloat16
EXP = mybir.ActivationFunctionType.Exp
BF = ml_dtypes.bfloat16


def make_plan(mask):
    """Per-unit key-block schedule derived from the [1,1,N,N] mask.

    units[u] = {kb: {"c0": first valid local q col (128-aligned),
                     "pats": [(qb_local, pattern_idx), ...]}}
    """
    m = np.asarray(mask).reshape(N, N)
    patterns = []
    pat_keys = {}
    units = []
    for u in range(NU):
        q0 = (u % 4) * UW
        sched = {}
        for kb in range(NBU):
            blk = m[q0:q0 + UW, kb * BT:(kb + 1) * BT]  # [512 q, 128 k]
            if not blk.any():
                continue
            rows = blk.any(axis=1)
            c0 = (int(np.argmax(rows)) // BT) * BT
            pats = []
            for qb in range(c0 // BT, UW // BT):
                b2 = blk[qb * BT:(qb + 1) * BT, :]
                if b2.all() or not b2.any():
                    continue
                pat = np.ascontiguousarray(b2.T.astype(np.float32))
                key = pat.tobytes()
                if key not in pat_keys:
                    pat_keys[key] = len(patterns)
                    patterns.append(pat)
                pats.append((qb, pat_keys[key]))
            sched[kb] = {"c0": c0, "pats": pats}
        units.append(sched)
    if not patterns:
        patterns.append(np.ones((BT, BT), np.float32))
    return {"patterns": np.stack(patterns).astype(BF), "units": units}


def build_nc(plan):
    nc = bacc.Bacc("TRN2", target_bir_lowering=False, debug=False,
                   num_devices=NCORES)
    n_pat = plan["patterns"].shape[0]
    units = plan["units"]

    xP = nc.dram_tensor("xP", [8 * BT, 8 * CH], BF16, kind="ExternalInput")
    wqP = nc.dram_tensor("wqP", [BT, 8 * BT], BF16, kind="ExternalInput")
    wkP = nc.dram_tensor("wkP", [BT, 8 * BT], BF16, kind="ExternalInput")
    wvP = nc.dram_tensor("wvP", [BT, 8 * BT], BF16, kind="ExternalInput")
    woP = nc.dram_tensor("woP", [BT, 8 * D], BF16, kind="ExternalInput")
    bo1 = nc.dram_tensor("bo1", [1, D], F32, kind="ExternalInput")
    pm = nc.dram_tensor("pm", [n_pat, BT, BT], BF16, kind="ExternalInput")
    ident = nc.dram_tensor("ident", [BT, BT], F32, kind="ExternalInput")
    y = nc.dram_tensor("y", [TPC, D], F32, kind="ExternalOutput")
    if DBG:
        d_qT = nc.dram_tensor("d_qT", [BT, TOK], BF16, kind="ExternalOutput")
        d_kT = nc.dram_tensor("d_kT", [BT, TOK], BF16, kind="ExternalOutput")
        d_v2 = nc.dram_tensor("d_v2", [BT, 32 * 130], BF16,
                              kind="ExternalOutput")
        d_pT = nc.dram_tensor("d_pT", [BT, 1024], BF16, kind="ExternalOutput")
        d_pO = nc.dram_tensor("d_pO", [HD + 1, UW], F32,
                              kind="ExternalOutput")
        d_aS = nc.dram_tensor("d_aS", [BT, UW], BF16, kind="ExternalOutput")
        d_aT = nc.dram_tensor("d_aT", [BT, 8 * BT], BF16,
                              kind="ExternalOutput")

    with tile.TileContext(nc) as tc:
        with (
            tc.tile_pool(name="const", bufs=1) as cp,
            tc.tile_pool(name="big", bufs=1) as bigp,
            tc.tile_pool(name="psum", bufs=1, space="PSUM") as psum,
            tc.tile_pool(name="dram", bufs=1, space="DRAM") as dram,
        ):
            # weight/const tiles (DMA issue order is tuned below: wq first,
            # then the first x chunk, then the rest)
            wq_all = cp.tile([BT, 8 * BT], BF16, name="wq_all")
            wk_all = cp.tile([BT, 8 * BT], BF16, name="wk_all")
            wv_all = cp.tile([BT, 8 * BT], BF16, name="wv_all")
            wq = [wq_all[:, e * BT:(e + 1) * BT] for e in range(8)]
            wk = [wk_all[:, e * BT:(e + 1) * BT] for e in range(8)]
            wv = [wv_all[:, e * BT:(e + 1) * BT] for e in range(8)]
            wo_all = cp.tile([BT, 8 * D], BF16, name="wo_all")
            pmt = [cp.tile([BT, BT], BF16, name=f"pmt{i}") for i in range(n_pat)]
            identt = cp.tile([BT, BT], F32, name="identt")
            bot = cp.tile([1, D], F32, name="bot")
            bobc = cp.tile([BT, D], F32, name="bobc")

            # ---- warm-up collective, fired immediately: the CC stream's
            # init barrier completes only once ALL cores reach their first
            # trigger, so firing at t~2us absorbs the core-launch skew
            # before the real exchanges need the stream ----
            wa_sb = cp.tile([BT, 4], F32, name="wa_sb")
            nc.vector.memset(wa_sb[:], 1.0)
            wa_in = dram.tile([BT, 4], F32, name="wa_in")
            wa_out = dram.tile([BT * NCORES, 4], F32, name="wa_out",
                               addr_space="Shared")
            nc.gpsimd.dma_start(wa_in[:], wa_sb[:])
            nc.gpsimd.collective_compute(
                "AllGather", mybir.AluOpType.bypass,
                ins=[wa_in.opt()], outs=[wa_out.opt()],
                replica_groups=[list(range(NCORES))])

            # A collective trigger freezes the Pool queue until the CC
            # stream accepts it. Early triggers are deferred one unit so
            # the Pool-hosted reciprocal broadcasts of units 1-3 are not
            # frozen behind them while the stream warms up.
            def trigger_a2a(u):
                nc.gpsimd.collective_compute(
                    "AllToAll", mybir.AluOpType.bypass,
                    ins=[a2a_in[u].opt()], outs=[a2a_out[u].opt()],
                    replica_groups=[list(range(NCORES))])

            # per-unit exchange buffers (units 4-7); sections 0-1 exchange
            # with one double-size collective each to relieve the CC stream
            # while it warms up
            a2a_in = [dram.tile([NCORES * BT, HD], BF16, name=f"a2a_in{u}")
                      for u in range(NU)]
            a2a_out = [dram.tile([NCORES * BT, HD], BF16, name=f"a2a_out{u}")
                       for u in range(NU)]
            a2s_in = [dram.tile([NCORES * 2 * BT, HD], BF16,
                                name=f"a2s_in{s}") for s in range(2)]
            a2s_out = [dram.tile([NCORES * 2 * BT, HD], BF16,
                                 name=f"a2s_out{s}") for s in range(2)]

            def trigger_a2s(s):
                nc.gpsimd.collective_compute(
                    "AllToAll", mybir.AluOpType.bypass,
                    ins=[a2s_in[s].opt()], outs=[a2s_out[s].opt()],
                    replica_groups=[list(range(NCORES))])

            qTt = bigp.tile([BT, TOK], BF16, name="qTt")
            kTt = bigp.tile([BT, TOK], BF16, name="kTt")
            # V token-major: [128 tok, 130] per block: [64 h0 | 1 | 64 h1 | 1]
            v2 = bigp.tile([BT, 32 * 130], BF16, name="v2")
            nc.vector.memset(
                v2[:].rearrange("p (t j c) -> p t j c", t=32, j=2)
                [:, :, :, HD:HD + 1], 1.0)

            with (
                tc.tile_pool(name="xp", bufs=2) as xp,
                tc.tile_pool(name="ptp", bufs=3) as ptp,
                tc.tile_pool(name="workp", bufs=2) as workp,
            ):
                xtiles = {}

                def load_x(ch):
                    xt_all = xp.tile([BT, 8 * CH], BF16, name="xt", tag="xt")
                    for q in range(4):
                        cs = slice(1024 * q, 1024 * (q + 1))
                        nc.sync.dma_start(
                            xt_all[:, cs],
                            xP.ap()[ch * BT:(ch + 1) * BT, cs])
                    xtiles[ch] = xt_all

                def emit_qkv_chunk(ch):
                    xt_all = xtiles.pop(ch)
                    xt = [xt_all[:, e * CH:(e + 1) * CH] for e in range(8)]
                    # Q and K into one [128, 1024] PSUM tile (psS tag)
                    psqk = psum.tile([BT, 1024], F32, name="psS", tag="psS",
                                     bufs=2)
                    for (qk, wt) in ((0, wq), (1, wk)):
                        for e in range(8):
                            nc.tensor.matmul(psqk[:, 512 * qk:512 * (qk + 1)],
                                             wt[e], xt[e],
                                             start=(e == 0), stop=(e == 7))
                    for (qk, dst) in ((0, qTt), (1, kTt)):
                        nc.vector.tensor_copy(dst[:, ch * CH:(ch + 1) * CH],
                                              psqk[:, 512 * qk:512 * (qk + 1)])
                    # V channel-major (512-wide streams), then token-major
                    # via PE transposes (bf16, 1 cycle/row)
                    psv = psum.tile([BT, 1024], F32, name="psS", tag="psS",
                                    bufs=2)
                    for e in range(8):
                        nc.tensor.matmul(psv[:, 0:CH], wv[e], xt[e],
                                         start=(e == 0), stop=(e == 7))
                    vTs = workp.tile([BT, CH], F32, name="vTs", tag="vTs")
                    nc.vector.tensor_copy(vTs[:], psv[:, 0:CH])
                    pst = psum.tile([BT, 1024], F32, name="psS", tag="psS",
                                    bufs=2)
                    for t in range(4):
                        nc.tensor.transpose(pst[:, t * BT:(t + 1) * BT],
                                            vTs[:, t * BT:(t + 1) * BT],
                                            identt[:])
                    for t in range(4):
                        tb = ch * 4 + t
                        nc.vector.tensor_copy(
                            v2[:, 130 * tb:130 * tb + 130]
                            .rearrange("p (j c) -> p j c", j=2)[:, :, 0:HD],
                            pst[:, t * BT:(t + 1) * BT]
                            .rearrange("p (j c) -> p j c", j=2))

                def emit_exchange(u, attnS):
                    if u < 4:
                        # a2s_in[(r, h, jp), c] = attnS[jp, 64 r + c]
                        sec, h = u // 2, u % 2
                        nc.sync.dma_start(
                            a2s_in[sec][:]
                            .rearrange("(r g p) c -> g p r c", g=2, p=BT)[h],
                            attnS[:].rearrange("p (r c) -> p r c", r=NCORES))
                    else:
                        # a2a_in[(r, jp), c] = attnS[jp, 64 r + c]
                        nc.sync.dma_start(
                            a2a_in[u][:].rearrange("(r p) c -> p r c", p=BT),
                            attnS[:].rearrange("p (r c) -> p r c", r=NCORES))
                    if u == 2:
                        trigger_a2s(0)
                    elif u == 3:
                        trigger_a2s(1)
                    elif u >= 4:
                        trigger_a2a(u)

                def emit_outproj(sec, halves=(0, 1), rows=None):
                    # aT[p, 128 i + 64 h + c] <- exchanged attention
                    aT = workp.tile([BT, 8 * BT], BF16, name=f"aT{halves[0]}",
                                    tag=f"aT{halves[0]}")
                    for h in halves:
                        dst = aT[:].rearrange("p (i g c) -> p i g c", i=8,
                                              g=2, c=HD)[:, :, h, :]
                        if sec < 2:
                            nc.sync.dma_start(
                                dst,
                                a2s_out[sec][:]
                                .rearrange("(i g p) c -> g p i c", g=2,
                                           p=BT)[h])
                        else:
                            nc.sync.dma_start(
                                dst,
                                a2a_out[2 * sec + h][:]
                                .rearrange("(i p) c -> p i c", p=BT))
                    if DBG and sec == 0 and halves == (0, 1):
                        nc.sync.dma_start(d_aT.ap(), aT[:])
                    rs = rows if rows is not None else slice(0, BT)
                    aTs = [aT[:, i * BT:(i + 1) * BT][:, rs] for i in range(8)]
                    nr = rs.stop - rs.start
                    psY = psum.tile([BT, 1024], F32, name="psS", tag="psS",
                                    bufs=2)
                    for oc in range(2):
                        for i in range(8):
                            nc.tensor.matmul(
                                psY[0:nr, oc * 512:(oc + 1) * 512], aTs[i],
                                wo_all[:, i * D + oc * 512:
                                       i * D + (oc + 1) * 512],
                                start=(i == 0), stop=(i == 7))
                    for oc in range(2):
                        ysb = workp.tile([BT, 512], F32, name="ysb", tag="ysb")
                        nc.vector.tensor_add(
                            ysb[0:nr, :], psY[0:nr, oc * 512:(oc + 1) * 512],
                            bobc[rs, oc * 512:(oc + 1) * 512])
                        nc.sync.dma_start(
                            y.ap()[BT * sec + rs.start:BT * sec + rs.stop,
                                   oc * 512:(oc + 1) * 512],
                            ysb[0:nr, :])

                nc.scalar.dma_start(wq_all[:], wqP.ap())
                load_x(0)
                nc.scalar.dma_start(wk_all[:], wkP.ap())
                nc.scalar.dma_start(wv_all[:], wvP.ap())
                for i in range(n_pat):
                    nc.scalar.dma_start(pmt[i][:], pm.ap()[i])
                nc.scalar.dma_start(identt[:], ident.ap())
                nc.scalar.dma_start(bot[:], bo1.ap())
                nc.gpsimd.partition_broadcast(bobc[:], bot[:])
                for q in range(4):
                    cs = slice(2048 * q, 2048 * (q + 1))
                    nc.scalar.dma_start(wo_all[:, cs], woP.ap()[:, cs])

                for u in range(NU):
                    b, qh = u // 4, u % 4
                    if u + 1 < NU:
                        load_x(u + 1)
                    emit_qkv_chunk(u)

                    sched = units[u]
                    kbs = sorted(sched.keys())
                    psO = [psum.tile([HD + 1, UW], F32, name=f"psO{j}",
                                     tag=f"psO{j}", bufs=2)
                           for j in range(HPC)]
                    qcols = slice(b * N + qh * UW, b * N + (qh + 1) * UW)
                    diag = [kb for kb in kbs if sched[kb]["pats"]
                            or sched[kb]["c0"] > 0]
                    full = [kb for kb in kbs if kb not in diag]

                    def emit_scores(kb, tag):
                        kcols = slice(b * N + kb * BT, b * N + (kb + 1) * BT)
                        c0 = sched[kb]["c0"]
                        psS = psum.tile([BT, 1024], F32, name="psS", tag="psS",
                                        bufs=2)
                        pT = ptp.tile([BT, 1024], BF16, name="pT", tag=tag,
                                      bufs=4 if tag == "pTd" else 3)
                        for j in range(HPC):
                            ks = slice(HD * j, HD * (j + 1))
                            nc.tensor.matmul(
                                psS[:, 512 * j + c0:512 * (j + 1)],
                                kTt[ks, kcols],
                                qTt[ks, qcols][:, c0:UW],
                                start=True, stop=True,
                                tile_position=(HD * j, 0))
                        # one exp for both heads (3D AP, suffix width)
                        nc.scalar.activation(
                            pT[:].rearrange("p (j c) -> p j c",
                                            j=2)[:, :, c0:UW],
                            psS[:].rearrange("p (j c) -> p j c",
                                             j=2)[:, :, c0:UW],
                            EXP, scale=0.125)
                        # partial-block masking (per head, on DVE)
                        for (qb, pidx) in sched[kb]["pats"]:
                            for j in range(HPC):
                                slc = pT[:, 512 * j + qb * BT:
                                         512 * j + (qb + 1) * BT]
                                nc.vector.tensor_mul(slc, slc, pmt[pidx][:])
                        if DBG and u == 1 and kb == 0:
                            nc.sync.dma_start(d_pT.ap(), pT[:])
                        return pT

                    def emit_av(kb, pT, start, stop):
                        c0 = sched[kb]["c0"]
                        for j in range(HPC):
                            vt = v2[:, 130 * (b * NBU + kb) + 65 * j:
                                    130 * (b * NBU + kb) + 65 * (j + 1)]
                            nc.tensor.matmul(
                                psO[j][:, c0:UW], vt,
                                pT[:, 512 * j + c0:512 * (j + 1)],
                                start=start, stop=stop,
                                skip_group_check=True)

                    if full:
                        # diag scores first (their exp+mask pipeline runs
                        # while the PE chews the full blocks), diag AV last
                        dpts = [emit_scores(kb, "pTd") for kb in diag]
                        for (n, kb) in enumerate(full):
                            pT = emit_scores(kb, "pT")
                            emit_av(kb, pT, start=(n == 0), stop=False)
                        for (n, kb) in enumerate(diag):
                            emit_av(kb, dpts[n], start=False,
                                    stop=(n == len(diag) - 1))
                    else:
                        # first unit of a batch: all blocks diagonal
                        for (n, kb) in enumerate(diag):
                            pT = emit_scores(kb, "pT")
                            emit_av(kb, pT, start=(n == 0),
                                    stop=(n == len(diag) - 1))

                    if u == NU - 1:
                        # final section's first-half projection: emitted
                        # before the last staging so its unstage DMA is not
                        # queued behind it on the sync queue, and the PE
                        # fills the last collective's window
                        emit_outproj(3, halves=(0,), rows=slice(0, HD))

                    # normalize + stage + exchange
                    if DBG and u == 1:
                        pOsb = workp.tile([HD + 1, UW], F32, name="pOsb",
                                          tag="pOsb")
                        nc.vector.tensor_copy(pOsb[:], psO[0][:])
                        nc.sync.dma_start(d_pO.ap(), pOsb[:])
                    # copy unnormalized attention out first (releases psO
                    # without waiting on the Pool-hosted broadcast), then
                    # scale in place
                    attnS = workp.tile([BT, UW], BF16, name="attnS",
                                       tag="attnS")
                    for j in range(HPC):
                        dn = workp.tile([1, UW], F32, name=f"dn{j}",
                                        tag=f"dn{j}")
                        nc.vector.tensor_copy(dn[:], psO[j][HD:HD + 1, :])
                        rcp = workp.tile([1, UW], F32, name=f"rcp{j}",
                                         tag=f"rcp{j}")
                        nc.vector.reciprocal_approx_fast(rcp[:], dn[:])
                        rbc = workp.tile([HD, UW], F32, name=f"rbc{j}",
                                         tag=f"rbc{j}")
                        nc.gpsimd.partition_broadcast(rbc[:], rcp[:])
                        nc.vector.tensor_mul(attnS[HD * j:HD * (j + 1), :],
                                             psO[j][0:HD, :], rbc[:])
                    if DBG and u == 1:
                        nc.sync.dma_start(d_aS.ap(), attnS[:])
                    emit_exchange(u, attnS)
                    if u == 4:
                        emit_outproj(0)
                    elif u == 5:
                        emit_outproj(1)
                    elif u == 6:
                        emit_outproj(2)

                emit_outproj(3, halves=(1,), rows=slice(HD, BT))
                if DBG:
                    nc.sync.dma_start(d_qT.ap(), qTt[:])
                    nc.sync.dma_start(d_kT.ap(), kTt[:])
                    nc.sync.dma_start(d_v2.ap(), v2[:])
    nc.compile()
    return nc


_CACHE = {}


def _get_nc(plan_key, mask):
    if plan_key not in _CACHE:
        _CACHE[plan_key] = build_nc(make_plan(mask))
    return _CACHE[plan_key]


def _sb_layout(wT):
    """[8*128, C] row-block-major -> [128, 8*C] SBUF layout (contiguous)."""
    g, c = wT.shape[0] // BT, wT.shape[1]
    return np.ascontiguousarray(
        wT.reshape(g, BT, c).transpose(1, 0, 2).reshape(BT, g * c))


def _prep_inputs(x, mask, Wq, Wk, Wv, Wo, bo, plan):
    xT = x.reshape(TOK, D).T.astype(BF)          # [1024, 4096]
    xP = np.ascontiguousarray(
        xT.reshape(8, BT, 8, CH).transpose(2, 1, 0, 3).reshape(8 * BT, 8 * CH))
    woT = np.ascontiguousarray(Wo.T).astype(BF)
    bo1 = np.ascontiguousarray(bo[None, :]).astype(np.float32)
    woP = _sb_layout(woT)
    in_maps = []
    for c in range(NCORES):
        rows = slice(BT * c, BT * (c + 1))
        in_maps.append({
            "xP": xP,
            "wqP": _sb_layout(Wq[rows].T.astype(BF)),
            "wkP": _sb_layout(Wk[rows].T.astype(BF)),
            "wvP": _sb_layout(Wv[rows].T.astype(BF)),
            "woP": woP,
            "bo1": bo1,
            "pm": plan["patterns"],
            "ident": np.eye(BT, dtype=np.float32),
        })
    return in_maps


def run(inputs, trace=False, **kw):
    x = np.asarray(inputs["x"], np.float32)
    mask = np.asarray(inputs["mask"])
    plan_key = mask.tobytes()
    nc = _get_nc(plan_key, mask)
    plan = make_plan(mask)
    in_maps = _prep_inputs(x, mask, np.asarray(inputs["Wq"], np.float32),
                           np.asarray(inputs["Wk"], np.float32),
                           np.asarray(inputs["Wv"], np.float32),
                           np.asarray(inputs["Wo"], np.float32),
                           np.asarray(inputs["bo"], np.float32), plan)
    res = run_bass_kernel_spmd(nc, in_maps, core_ids=list(range(NCORES)),
                               trace=trace, **kw)
    # unshard: core c, section sec, half h, row i ->
    #   global token sec*1024 + 512 h + 64 c + i
    out = np.empty((TOK, D), np.float32)
    for c in range(NCORES):
        yc = res.results[c]["y"]
        for sec in range(4):
            for h in range(2):
                out[sec * 1024 + 512 * h + HD * c:
                    sec * 1024 + 512 * h + HD * (c + 1)] = \
                    yc[BT * sec + HD * h: BT * sec + HD * (h + 1)]
    return out.reshape(B, N, D), res


def kernel(**inputs):
    out, _ = run(inputs, trace=False)
    return out
